# revision 12
# baseline (speedup 1.0000x reference)
"""Trainium2 Bass kernel for the argmax-distance-weighted loss.

loss = sum_b sum_{j,k} ((jstar_b - j)^2 + (kstar_b - k)^2) * t[b,j,k]
where (jstar_b, kstar_b) is the (first-occurrence) argmax location of t[b].

Decomposition used per batch b:
    loss_b = (jstar^2 + kstar^2)*S - 2*jstar*Sj - 2*kstar*Sk + Sj2 + Sk2
with S    = sum t[b]
     Sj   = sum_j j   * rowsum[b, j]      rowsum[b,j] = sum_k t[b,j,k]
     Sj2  = sum_j j^2 * rowsum[b, j]
     Sk   = sum_k k   * colsum[b, k]      colsum[b,k] = sum_j t[b,j,k]
     Sk2  = sum_k k^2 * colsum[b, k]

Device (8 NeuronCores, data-parallel over batch): per 128-batch tile the
DVE does three full reduction passes (rowsum, colsum contiguous/strided,
rowmax) plus tiny fused weighted reductions, emitting 8 moments per batch.
jstar is recovered exactly (first row whose rowmax equals the batch max).
Host: gathers row jstar per batch (64 floats) to resolve kstar with exact
first-occurrence semantics, then evaluates the closed form and sums.
"""

import os
import sys

import numpy as np

try:
    import concourse.bass as bass
except ModuleNotFoundError:  # make concourse importable in a bare container
    for _p in ("/opt/trn_rl_repo", "/root/.axon_site/_ro/trn_rl_repo"):
        if os.path.isdir(_p) and _p not in sys.path:
            sys.path.insert(0, _p)
    import concourse.bass as bass

import concourse.mybir as mybir
from concourse.bass_utils import run_bass_kernel_spmd
from concourse.tile import TileContext
# --- workaround: this walrus build encodes only ONE sync-wait per TPB ---
# instruction. Tile attaches several waits to one instruction (tail drain,
# DMA copies, ...), which codegen rejects with "Too many sync wait
# commands". Post-pass: hoist all but the last wait of each instruction
# into standalone same-engine NoOps placed immediately before it.


def _split_multiwait_instructions(nc: bass.Bass) -> None:
    # (bb, inst-name) pairs needing surgery
    targets = []
    for fn in nc.m.functions:
        for bb in fn.blocks:
            for inst in bb.instructions:
                si = inst.sync_info
                if si is not None and len(si.on_wait) > 1:
                    targets.append((bb, inst.name))
    if not targets:
        return

    moved_nop_names: set[str] = set()
    plan: dict[str, list] = {}  # target-inst-name -> nop instructions
    for bb, iname in targets:
        inst = next(i for i in bb.instructions if i.name == iname)
        waits = list(inst.sync_info.on_wait)
        inst.sync_info.on_wait = waits[-1:]
        nops = []
        for w in waits[:-1]:
            bi = nc.engines[inst.engine].nop(nofuse=True, hint="split_wait")
            bi.ins.sync_info = mybir.SyncInfo(on_wait=[w], on_update=[])
            nops.append(bi.ins)
            moved_nop_names.add(bi.ins.name)
        plan[iname] = nops

    # relocate the nops to sit immediately before their target instruction
    for fn in nc.m.functions:
        for bb in fn.blocks:
            insts = list(bb.instructions)
            kept = [i for i in insts if i.name not in moved_nop_names]
            out: list = []
            changed = len(kept) != len(insts)
            for inst in kept:
                if inst.name in plan:
                    out.extend(plan[inst.name])
                    changed = True
                out.append(inst)
            if changed:
                bb.instructions = out

B, H, W = 8192, 64, 64
NCORES = 8
P = 128  # SBUF partitions

F32 = mybir.dt.float32
Alu = mybir.AluOpType
Ax = mybir.AxisListType

# output layout: quantity-major [P, NQ, ntiles]
Q_M, Q_S, Q_SJ, Q_SJ2, Q_SK, Q_SK2, Q_RJ = range(7)
NQ = 7


def build(bpc: int, repeats: int = 1, gp: bool = True) -> bass.Bass:
    """Build the per-core Bass program for `bpc` batches per core.

    `repeats` re-runs the whole pipeline N times in one program — used only
    for timing (slope method cancels the host dispatch overhead).
    `gp` offloads the colsum fold tree and the elementwise muls to GpSimd so
    the DVE runs almost only dedicated-port single-src reductions."""
    ntiles = bpc // P
    assert ntiles * P == bpc
    NT = ntiles

    nc = bass.Bass()
    x = nc.declare_dram_parameter("x", [bpc, H, W], F32, isOutput=False)
    wc = nc.declare_dram_parameter("wconsts", [3, NT, W], F32, isOutput=False)
    out = nc.declare_dram_parameter("moments", [P, NQ * NT], F32, isOutput=True)

    with TileContext(nc) as tc:
        with (
            tc.tile_pool(name="xpool", bufs=3) as xpool,
            tc.tile_pool(name="consts", bufs=1) as cpool,
            tc.tile_pool(name="inter", bufs=1) as ipool,
        ):
            # broadcast weight constants [3, NT, W] across all partitions
            wtile = cpool.tile([P, 3, NT, W], F32)
            wc_ap = wc[:, :, :]
            bcast = bass.AP(
                tensor=wc_ap.tensor,
                offset=wc_ap.offset,
                ap=[[0, P]] + list(wc_ap.ap),
            )
            nc.sync.dma_start(out=wtile, in_=bcast)
            w1 = wtile[:, 0, :, :]  # [P, NT, W] = j (0..63), tiled per tile
            wr = wtile[:, 2, :, :]  # [P, NT, W] = 64-j

            rs_all = ipool.tile([P, NT, W], F32)
            cs_all = ipool.tile([P, NT, W], F32)
            rm_all = ipool.tile([P, NT, W], F32)
            scrA = ipool.tile([P, NT, W], F32)
            scrB = ipool.tile([P, NT, W], F32)
            outq = ipool.tile([P, NQ * NT], F32)

            def O(q):
                return outq[:, q * NT : (q + 1) * NT]

            eng = nc.gpsimd if gp else nc.vector

            for rep in range(repeats):
                for t in range(ntiles):
                    xt = xpool.tile([P, H, W], F32)
                    nc.sync.dma_start(out=xt, in_=x[t * P : (t + 1) * P, :, :])

                    # full-data passes (DVE, single-src, dedicated port)
                    nc.vector.tensor_reduce(
                        out=rs_all[:, t, :], in_=xt[:, :, :], axis=Ax.X, op=Alu.add
                    )
                    nc.vector.tensor_reduce(
                        out=rm_all[:, t, :], in_=xt[:, :, :], axis=Ax.X, op=Alu.max
                    )
                    if gp:
                        # colsum over j as a fold tree on flat contiguous
                        # halves: out[i] = in[i] + in[i+half] (k stays the
                        # innermost 64)
                        xf = xt[:, :, :].rearrange("p a b -> p (a b)")
                        fold = xpool.tile([P, 4096], F32, tag="fold")
                        seg = [(0, 2048), (2048, 1024), (3072, 512),
                               (3584, 256), (3840, 128)]
                        src, src_off = xf, 0
                        for (dst_off, dst_n) in seg:
                            nc.gpsimd.tensor_tensor(
                                out=fold[:, dst_off : dst_off + dst_n],
                                in0=src[:, src_off : src_off + dst_n],
                                in1=src[:, src_off + dst_n : src_off + 2 * dst_n],
                                op=Alu.add,
                            )
                            src, src_off = fold, dst_off
                        nc.gpsimd.tensor_tensor(
                            out=cs_all[:, t, :], in0=fold[:, 3840:3904],
                            in1=fold[:, 3904:3968], op=Alu.add,
                        )
                    else:
                        xk = xt[:, :, :].rearrange("p j k -> p k j")
                        nc.vector.tensor_reduce(
                            out=cs_all[:, t, :], in_=xk, axis=Ax.X, op=Alu.add
                        )

                # batched epilogue over all tiles (free dim NT*W = small)
                nc.vector.tensor_reduce(out=O(Q_M), in_=rm_all[:, :, :], axis=Ax.X, op=Alu.max)
                nc.vector.tensor_reduce(out=O(Q_S), in_=rs_all[:, :, :], axis=Ax.X, op=Alu.add)
                eng.tensor_tensor(out=scrA, in0=rs_all, in1=w1, op=Alu.mult)
                nc.vector.tensor_reduce(out=O(Q_SJ), in_=scrA[:, :, :], axis=Ax.X, op=Alu.add)
                eng.tensor_tensor(out=scrB, in0=scrA, in1=w1, op=Alu.mult)
                nc.vector.tensor_reduce(out=O(Q_SJ2), in_=scrB[:, :, :], axis=Ax.X, op=Alu.add)
                eng.tensor_tensor(out=scrA, in0=cs_all, in1=w1, op=Alu.mult)
                nc.vector.tensor_reduce(out=O(Q_SK), in_=scrA[:, :, :], axis=Ax.X, op=Alu.add)
                eng.tensor_tensor(out=scrB, in0=scrA, in1=w1, op=Alu.mult)
                nc.vector.tensor_reduce(out=O(Q_SK2), in_=scrB[:, :, :], axis=Ax.X, op=Alu.add)
                # jstar: ge = (rm >= M) * (64-j); rj = max; jstar = 64 - rj
                mb = O(Q_M).unsqueeze(2).to_broadcast([P, NT, W])
                nc.vector.tensor_tensor(out=scrB, in0=rm_all, in1=mb, op=Alu.is_ge)
                eng.tensor_tensor(out=scrA, in0=scrB, in1=wr, op=Alu.mult)
                nc.vector.tensor_reduce(out=O(Q_RJ), in_=scrA[:, :, :], axis=Ax.X, op=Alu.max)

            nc.sync.dma_start(out=out[:, :], in_=outq)

    _split_multiwait_instructions(nc)
    return nc


_cache: dict[int, bass.Bass] = {}


def _get(bpc: int) -> bass.Bass:
    if bpc not in _cache:
        _cache[bpc] = build(bpc)
    return _cache[bpc]


def _wconsts(ntiles: int) -> np.ndarray:
    j = np.arange(W, dtype=np.float32)
    base = np.stack([j, j * j, (W - j).astype(np.float32)])  # [3, W]
    return np.repeat(base[:, None, :], ntiles, axis=1)  # [3, NT, W]


def _prepare(tensor: np.ndarray):
    t = np.ascontiguousarray(np.asarray(tensor), dtype=np.float32)
    bt = t.shape[0]
    bpc = bt // NCORES
    nc = _get(bpc)
    wc = _wconsts(bpc // P)
    in_maps = [
        {"x": t[c * bpc : (c + 1) * bpc], "wconsts": wc} for c in range(NCORES)
    ]
    return nc, in_maps, t


def _postprocess(t: np.ndarray, results: list[dict]) -> np.ndarray:
    bt = t.shape[0]
    bpc = bt // NCORES
    nt = bpc // P
    ms = []
    for c in range(NCORES):
        m = results[c]["moments"].reshape(P, NQ, nt)
        ms.append(m.transpose(2, 0, 1).reshape(bpc, NQ))  # batch-major
    m = np.concatenate(ms, 0).astype(np.float64)  # [B, NQ]

    S = m[:, Q_S]
    Sj = m[:, Q_SJ]
    Sj2 = m[:, Q_SJ2]
    Sk = m[:, Q_SK]
    Sk2 = m[:, Q_SK2]
    jstar = np.rint(W - m[:, Q_RJ]).astype(np.int64)

    # resolve kstar with exact first-occurrence semantics on the argmax row
    rows = t[np.arange(bt), jstar, :]  # [B, W]
    mrow = rows.max(axis=1)
    kstar = (rows == mrow[:, None]).argmax(axis=1)

    js = jstar.astype(np.float64)
    ks = kstar.astype(np.float64)
    loss = ((js * js + ks * ks) * S - 2.0 * js * Sj - 2.0 * ks * Sk + Sj2 + Sk2).sum()
    return np.asarray([loss], dtype=np.float32)


def kernel(tensor: np.ndarray) -> np.ndarray:
    nc, in_maps, t = _prepare(tensor)
    res = run_bass_kernel_spmd(nc, in_maps, list(range(NCORES)))
    return _postprocess(t, res.results)
